# revision 8
# baseline (speedup 1.0000x reference)
"""Distributed attention-layer kernel for 8 TRN2 NeuronCores.

Reference computation (per batch element b):
    Q = Wq @ x[b]; K = Wk @ x[b]; V = Wv @ x[b]
    S = Q^T K  (no scaling);  A = softmax(S, axis=keys)
    out[b] = V @ A^T          # [COUT, N]

Sharding: core i handles (b = i//2, query half h = i%2). The full
attention row block [2048 q x 4096 keys] stays local; no collectives.

Kernel algebra (per core):
    M^T = Wk^T Wq                       (128x128, one matmul)
    Z   = M x[b]   = (M^T)^T x[b]       [128, 4096]
    S^T[m,q] = sum_i Z[i,m] x[i,q]      -> matmul(lhsT=Z_chunk, rhs=xq), f32r
    P = exp(S^T)                        (ScalarE, PSUM->SBUF, bf16 out;
                                         no max-subtraction: max |S| ~ 67)
    num[o,q] = sum_m V^T[m,o] P[m,q]    -> bf16 PSUM-accumulated matmuls
    den[q]   = sum_m P[m,q]             -> P chunks pre-summed on DVE+GpSimd
                                           (4 bf16 accumulators), then one
                                           ones-vector matmul per supertile
    out = num * (1/den broadcast)       (broadcast via rank-1 matmul)

S^T runs in float32r (1 cycle/row at free dim >= 512, ~19-bit mantissa);
the post-exp path runs in bf16 (linear error only; total ~3e-3).
"""

import numpy as np

import concourse.bass as bass
import concourse.bacc as bacc
import concourse.mybir as mybir
from concourse.tile import TileContext
from concourse.bass_utils import run_bass_kernel_spmd
from concourse.masks import make_identity

B, CIN, N = 4, 128, 4096
CKEY, COUT = 64, 128
NCORES = 8
NQ = N // 2            # queries per core
QT = 512               # query supertile (PSUM bank width in f32)
NST = NQ // QT         # 4 supertiles
MC = 128               # key-chunk size (partition dim)
NMC = N // MC          # 32 key chunks
GRP = 3                # key chunks per exp group ([128, 1536] = 3 banks)
NACC = 4               # den accumulators (chunk c -> acc c % NACC)

F32 = mybir.dt.float32
F32R = mybir.dt.float32r
BF16 = mybir.dt.bfloat16
EXP = mybir.ActivationFunctionType.Exp
ADD = mybir.AluOpType.add


def _build() -> bacc.Bacc:
    nc = bacc.Bacc()
    xq = nc.declare_dram_parameter("xq", [CIN, NQ], F32, isOutput=False)
    xk = nc.declare_dram_parameter("xk", [CIN, N], F32, isOutput=False)
    wq = nc.declare_dram_parameter("wq", [CKEY, CIN], F32, isOutput=False)
    wk = nc.declare_dram_parameter("wk", [CKEY, CIN], F32, isOutput=False)
    wv = nc.declare_dram_parameter("wv", [COUT, CIN], F32, isOutput=False)
    out = nc.declare_dram_parameter("out", [COUT, NQ], F32, isOutput=True)

    with TileContext(nc) as tc:
        with (
            tc.tile_pool(name="big", bufs=1) as big,
            tc.tile_pool(name="ptp", bufs=5) as ptp,
            tc.tile_pool(name="accp", bufs=2) as accp,
            tc.tile_pool(name="outp", bufs=2) as outp,
            tc.tile_pool(name="stp", bufs=2, space="PSUM") as stp,
            tc.tile_pool(name="avp", bufs=1, space="PSUM") as avp,
        ):
            # ---- loads (weights + queries first: they gate the preamble) ----
            wq_sb = big.tile([CKEY, CIN], F32)
            wk_sb = big.tile([CKEY, CIN], F32)
            wv_sb = big.tile([COUT, CIN], F32)
            nc.sync.dma_start(wq_sb[:], wq[:])
            nc.sync.dma_start(wk_sb[:], wk[:])
            nc.sync.dma_start(wv_sb[:], wv[:])
            xq_sb = big.tile([CIN, NQ], F32)
            nc.sync.dma_start(xq_sb[:, :NQ // 2], xq[:, :NQ // 2])
            nc.sync.dma_start(xq_sb[:, NQ // 2:], xq[:, NQ // 2:])
            xk_sb = big.tile([CIN, N], F32)
            NK4 = N // 4
            for qtr in range(4):
                nc.sync.dma_start(xk_sb[:, qtr * NK4:(qtr + 1) * NK4],
                                  xk[:, qtr * NK4:(qtr + 1) * NK4])

            # ---- f32r / bf16 working copies (DVE; legal rounding producers) ----
            xq_r = big.tile([CIN, NQ], F32R)
            nc.vector.tensor_copy(xq_r[:, :NQ // 2], xq_sb[:, :NQ // 2])
            nc.vector.tensor_copy(xq_r[:, NQ // 2:], xq_sb[:, NQ // 2:])
            xk_r = big.tile([CIN, N], F32R)
            xk_bf = big.tile([CIN, N], BF16)
            for qtr in range(4):
                sl = slice(qtr * NK4, (qtr + 1) * NK4)
                nc.vector.tensor_copy(xk_r[:, sl], xk_sb[:, sl])
                nc.gpsimd.tensor_copy(xk_bf[:, sl], xk_sb[:, sl])

            # zero-padded [128,128] weight blocks (contraction over CKEY=64);
            # memset/affine on f32r tiles fails the ISA check, so stage in f32
            wq_pad = big.tile([CIN, CIN], F32)
            wk_pad = big.tile([CIN, CIN], F32)
            nc.vector.memset(wq_pad[:], 0.0)
            nc.vector.memset(wk_pad[:], 0.0)
            nc.vector.tensor_copy(wq_pad[:CKEY, :], wq_sb[:])
            nc.vector.tensor_copy(wk_pad[:CKEY, :], wk_sb[:])
            wq_r = big.tile([CIN, CIN], F32R)
            wk_r = big.tile([CIN, CIN], F32R)
            nc.vector.tensor_copy(wq_r[:], wq_pad[:])
            nc.vector.tensor_copy(wk_r[:], wk_pad[:])
            wv_r = big.tile([COUT, CIN], F32R)
            nc.vector.tensor_copy(wv_r[:], wv_sb[:])

            ident_f = big.tile([CIN, CIN], F32)
            make_identity(nc, ident_f[:])
            ident_r = big.tile([CIN, CIN], F32R)
            nc.vector.tensor_copy(ident_r[:], ident_f[:])
            ones_col_f = big.tile([CIN, 1], F32)
            ones_row_f = big.tile([1, CIN], F32)
            nc.vector.memset(ones_col_f[:], 1.0)
            nc.vector.memset(ones_row_f[:], 1.0)
            ones_col = big.tile([CIN, 1], BF16)
            ones_row = big.tile([1, CIN], F32R)
            nc.vector.tensor_copy(ones_col[:], ones_col_f[:])
            nc.vector.tensor_copy(ones_row[:], ones_row_f[:])

            # ---- M^T = Wk^T Wq ----
            mt_ps = stp.tile([CIN, GRP * QT], F32, tag="ps", name="mt_ps")
            nc.tensor.matmul(mt_ps[:, :CIN], wk_r[:], wq_r[:], start=True, stop=True)
            mt_r = big.tile([CIN, CIN], F32R)
            nc.vector.tensor_copy(mt_r[:], mt_ps[:, :CIN])

            # ---- Wv^T via identity matmul; stored bf16 for the AV path ----
            wvt_ps = stp.tile([CIN, GRP * QT], F32, tag="ps", name="wvt_ps")
            nc.tensor.matmul(wvt_ps[:, :CIN], wv_r[:], ident_r[:], start=True, stop=True)
            wvt_bf = big.tile([CIN, COUT], BF16)
            nc.vector.tensor_copy(wvt_bf[:], wvt_ps[:, :CIN])

            # ---- Z = M @ x = (M^T)^T x  (f32r) ----
            z_r = big.tile([CIN, N], F32R)
            for j in range(4):
                zp = stp.tile([CIN, GRP * QT], F32, tag="ps", name="zp")
                lo = j * 2 * QT
                nc.tensor.matmul(zp[:, :QT], mt_r[:], xk_r[:, lo: lo + QT],
                                 start=True, stop=True)
                nc.tensor.matmul(zp[:, QT: 2 * QT], mt_r[:],
                                 xk_r[:, lo + QT: lo + 2 * QT],
                                 start=True, stop=True)
                nc.vector.tensor_copy(z_r[:, lo: lo + 2 * QT], zp[:, : 2 * QT])

            # ---- V^T chunks (bf16 matmuls; FWL-fast weight loads) ----
            vt_bf = big.tile([CIN, NMC, MC], BF16)
            for grp in range(NMC // 8):
                vp = stp.tile([CIN, GRP * QT], F32, tag="ps", name="vp")
                for k in range(8):
                    c = grp * 8 + k
                    nc.tensor.matmul(
                        vp[:, k * MC: (k + 1) * MC],
                        xk_bf[:, c * MC: (c + 1) * MC],
                        wvt_bf[:],
                        start=True, stop=True,
                    )
                nc.vector.tensor_copy(vt_bf[:, grp * 8: (grp + 1) * 8, :],
                                      vp[:, : 8 * MC])

            # ---- main loop over query supertiles ----
            groups = []
            c = 0
            while c < NMC:
                cnt = min(GRP, NMC - c)
                groups.append((c, cnt))
                c += cnt

            # den routing: chunks 0-20 -> DVE accumulators (7 each),
            # 21-29 -> GpSimd accumulator, 30-31 -> PE ones-matmuls (keeps
            # the end-of-supertile critical path short)
            DVE_HI = 21
            GPS_HI = 30

            for st in range(NST):
                q0 = st * QT
                xq_st = xq_r[:, q0: q0 + QT]
                # av[:, :QT] accumulates num[o,q]; av[:, QT:] later holds
                # den broadcast (bank B, written by den-mm then rb-mm)
                av = avp.tile([COUT, 2 * QT], F32, tag="av", name="av")
                accs = [accp.tile([MC, QT], BF16, name=f"acc{a}", tag=f"acc{a}")
                        for a in range(NACC)]
                seen = [0] * NACC
                for c0, cnt in groups:
                    ps = stp.tile([MC, GRP * QT], F32, tag="ps", name="ps")
                    for k in range(cnt):
                        nc.tensor.matmul(
                            ps[:, k * QT: (k + 1) * QT],
                            z_r[:, (c0 + k) * MC: (c0 + k + 1) * MC],
                            xq_st, start=True, stop=True)
                    pt = ptp.tile([MC, GRP * QT], BF16, tag="pt", name="pt")
                    nc.scalar.activation(pt[:, : cnt * QT], ps[:, : cnt * QT], EXP)
                    for k in range(cnt):
                        cc = c0 + k
                        nc.tensor.matmul(av[:, :QT], vt_bf[:, cc, :],
                                         pt[:, k * QT: (k + 1) * QT],
                                         start=(cc == 0), stop=(cc == NMC - 1))
                    for k in range(cnt):
                        cc = c0 + k
                        src = pt[:, k * QT: (k + 1) * QT]
                        if cc >= GPS_HI:
                            # tail chunks: reduce directly on PE into av bank B
                            nc.tensor.matmul(av[:1, QT:], ones_col[:], src,
                                             start=False, stop=(cc == NMC - 1))
                            continue
                        a = min(cc // 7, 2) if cc < DVE_HI else 3
                        eng = nc.gpsimd if a == 3 else nc.vector
                        if seen[a] == 0:
                            eng.tensor_copy(accs[a][:], src)
                        else:
                            eng.tensor_tensor(accs[a][:], accs[a][:], src, ADD)
                        seen[a] += 1
                    if c0 + cnt == GPS_HI:
                        # all accumulator chunks are in: combine and reduce
                        # over partitions into av bank B (opens the group
                        # the two PE tail ones-matmuls accumulate into)
                        acc01 = accp.tile([MC, QT], BF16, name="acc01", tag="acc01")
                        nc.gpsimd.tensor_tensor(acc01[:], accs[0][:], accs[1][:], ADD)
                        acc23 = accp.tile([MC, QT], BF16, name="acc23", tag="acc23")
                        nc.vector.tensor_tensor(acc23[:], accs[2][:], accs[3][:], ADD)
                        accf = accp.tile([MC, QT], BF16, name="accf", tag="accf")
                        nc.vector.tensor_tensor(accf[:], acc01[:], acc23[:], ADD)
                        nc.tensor.matmul(av[:1, QT:], ones_col[:], accf[:],
                                         start=True, stop=False)

                # normalize: out = num / den
                den_r = outp.tile([1, QT], F32R, name="den_r")
                nc.vector.tensor_copy(den_r[:], av[:1, QT:])
                # rank-1 matmul broadcasts den over all 128 partitions
                nc.tensor.matmul(av[:, QT:], ones_row[:], den_r[:],
                                 start=True, stop=True)
                rb_sb = outp.tile([COUT, QT], F32, name="rb_sb")
                nc.vector.reciprocal_approx_fast(rb_sb[:], av[:, QT:])
                o_sb = outp.tile([COUT, QT], F32, name="o_sb")
                nc.vector.tensor_tensor(o_sb[:], av[:, :QT], rb_sb[:],
                                        mybir.AluOpType.mult)
                nc.sync.dma_start(out[:, q0: q0 + QT], o_sb[:])

    nc.finalize()
    return nc


_NC_CACHE: list = []
LAST_RESULTS = None


def _get_nc() -> bacc.Bacc:
    if not _NC_CACHE:
        _NC_CACHE.append(_build())
    return _NC_CACHE[0]


def kernel(x, Wq, Wk, Wv, _trace=False):
    global LAST_RESULTS
    x = np.asarray(x, dtype=np.float32)
    wq = np.ascontiguousarray(np.asarray(Wq, dtype=np.float32))
    wk = np.ascontiguousarray(np.asarray(Wk, dtype=np.float32))
    wv = np.ascontiguousarray(np.asarray(Wv, dtype=np.float32))

    nc = _get_nc()
    in_maps = []
    for i in range(NCORES):
        b, h = divmod(i, 2)
        in_maps.append({
            "xq": np.ascontiguousarray(x[b][:, h * NQ: (h + 1) * NQ]),
            "xk": np.ascontiguousarray(x[b]),
            "wq": wq,
            "wk": wk,
            "wv": wv,
        })
    res = run_bass_kernel_spmd(nc, in_maps, core_ids=list(range(NCORES)),
                               trace=_trace)
    LAST_RESULTS = res
    out = np.empty((B, COUT, N), dtype=np.float32)
    for i in range(NCORES):
        b, h = divmod(i, 2)
        out[b][:, h * NQ: (h + 1) * NQ] = res.results[i]["out"]
    return out


# revision 10
# speedup vs baseline: 1.2018x; 1.2018x over previous
"""Distributed attention-layer kernel for 8 TRN2 NeuronCores.

Reference computation (per batch element b):
    Q = Wq @ x[b]; K = Wk @ x[b]; V = Wv @ x[b]
    S = Q^T K  (no scaling);  A = softmax(S, axis=keys)
    out[b] = V @ A^T          # [COUT, N]

Sharding: core i handles (b = i//2, query half h = i%2). The full
attention row block [2048 q x 4096 keys] stays local; no collectives.

Kernel algebra (per core):
    M^T = Wk^T Wq                       (128x128, one matmul)
    Z   = M x[b]   = (M^T)^T x[b]       [128, 4096]
    S^T[m,q] = sum_i Z[i,m] x[i,q]      -> matmul(lhsT=Z_chunk, rhs=xq), f32r
    P = exp(S^T)                        (ScalarE, PSUM->SBUF, bf16 out;
                                         no max-subtraction: max |S| ~ 67)
    num[o,q] = sum_m V^T[m,o] P[m,q]    -> bf16 PSUM-accumulated matmuls
    den[q]   = sum_m P[m,q]             -> P chunks pre-summed on DVE+GpSimd
                                           (4 bf16 accumulators), then one
                                           ones-vector matmul per supertile
    out = num * (1/den broadcast)       (broadcast via rank-1 matmul)

S^T runs in float32r (1 cycle/row at free dim >= 512, ~19-bit mantissa);
the post-exp path runs in bf16 (linear error only; total ~3e-3).
"""

import numpy as np

import concourse.bass as bass
import concourse.bacc as bacc
import concourse.mybir as mybir
from concourse.tile import TileContext
from concourse.bass_utils import run_bass_kernel_spmd
from concourse.masks import make_identity

B, CIN, N = 4, 128, 4096
CKEY, COUT = 64, 128
NCORES = 8
NQ = N // 2            # queries per core
QT = 512               # query supertile (PSUM bank width in f32)
NST = NQ // QT         # 4 supertiles
MC = 128               # key-chunk size (partition dim)
NMC = N // MC          # 32 key chunks
GRP = 3                # key chunks per exp group ([128, 1536] = 3 banks)
NACC = 4               # den accumulators (chunk c -> acc c % NACC)

F32 = mybir.dt.float32
F32R = mybir.dt.float32r
BF16 = mybir.dt.bfloat16
EXP = mybir.ActivationFunctionType.Exp
ADD = mybir.AluOpType.add


def _build() -> bacc.Bacc:
    nc = bacc.Bacc()
    xq = nc.declare_dram_parameter("xq", [CIN, NQ], F32, isOutput=False)
    xk = nc.declare_dram_parameter("xk", [CIN, N], F32, isOutput=False)
    wq = nc.declare_dram_parameter("wq", [CKEY, CIN], F32, isOutput=False)
    wk = nc.declare_dram_parameter("wk", [CKEY, CIN], F32, isOutput=False)
    wv = nc.declare_dram_parameter("wv", [COUT, CIN], F32, isOutput=False)
    out = nc.declare_dram_parameter("out", [COUT, NQ], F32, isOutput=True)

    with TileContext(nc) as tc:
        with (
            tc.tile_pool(name="big", bufs=1) as big,
            tc.tile_pool(name="ptp", bufs=5) as ptp,
            tc.tile_pool(name="accp", bufs=2) as accp,
            tc.tile_pool(name="outp", bufs=2) as outp,
            tc.tile_pool(name="stp", bufs=2, space="PSUM") as stp,
            tc.tile_pool(name="avp", bufs=1, space="PSUM") as avp,
        ):
            # ---- loads (weights + queries first: they gate the preamble) ----
            wq_sb = big.tile([CKEY, CIN], F32)
            wk_sb = big.tile([CKEY, CIN], F32)
            wv_sb = big.tile([COUT, CIN], F32)
            nc.sync.dma_start(wq_sb[:], wq[:])
            nc.sync.dma_start(wk_sb[:], wk[:])
            nc.sync.dma_start(wv_sb[:], wv[:])
            xq_sb = big.tile([CIN, NQ], F32)
            nc.sync.dma_start(xq_sb[:, :NQ // 2], xq[:, :NQ // 2])
            nc.sync.dma_start(xq_sb[:, NQ // 2:], xq[:, NQ // 2:])
            xk_sb = big.tile([CIN, N], F32)
            NK4 = N // 4
            for qtr in range(4):
                nc.sync.dma_start(xk_sb[:, qtr * NK4:(qtr + 1) * NK4],
                                  xk[:, qtr * NK4:(qtr + 1) * NK4])

            # ---- f32r / bf16 working copies (DVE; legal rounding producers) ----
            xq_r = big.tile([CIN, NQ], F32R)
            nc.vector.tensor_copy(xq_r[:, :NQ // 2], xq_sb[:, :NQ // 2])
            nc.vector.tensor_copy(xq_r[:, NQ // 2:], xq_sb[:, NQ // 2:])
            xk_r = big.tile([CIN, N], F32R)
            xk_bf = big.tile([CIN, N], BF16)
            for qtr in range(4):
                sl = slice(qtr * NK4, (qtr + 1) * NK4)
                nc.vector.tensor_copy(xk_r[:, sl], xk_sb[:, sl])
                nc.gpsimd.tensor_copy(xk_bf[:, sl], xk_sb[:, sl])

            # zero-padded [128,128] weight blocks (contraction over CKEY=64);
            # memset/affine on f32r tiles fails the ISA check, so stage in f32
            wq_pad = big.tile([CIN, CIN], F32)
            wk_pad = big.tile([CIN, CIN], F32)
            nc.vector.memset(wq_pad[:], 0.0)
            nc.vector.memset(wk_pad[:], 0.0)
            nc.vector.tensor_copy(wq_pad[:CKEY, :], wq_sb[:])
            nc.vector.tensor_copy(wk_pad[:CKEY, :], wk_sb[:])
            wq_r = big.tile([CIN, CIN], F32R)
            wk_r = big.tile([CIN, CIN], F32R)
            nc.vector.tensor_copy(wq_r[:], wq_pad[:])
            nc.vector.tensor_copy(wk_r[:], wk_pad[:])
            wv_r = big.tile([COUT, CIN], F32R)
            nc.vector.tensor_copy(wv_r[:], wv_sb[:])

            ident_f = big.tile([CIN, CIN], F32)
            make_identity(nc, ident_f[:])
            ident_r = big.tile([CIN, CIN], F32R)
            nc.vector.tensor_copy(ident_r[:], ident_f[:])
            ones_col_f = big.tile([CIN, 1], F32)
            ones_row_f = big.tile([1, CIN], F32)
            nc.vector.memset(ones_col_f[:], 1.0)
            nc.vector.memset(ones_row_f[:], 1.0)
            ones_col = big.tile([CIN, 1], BF16)
            ones_row = big.tile([1, CIN], F32R)
            nc.vector.tensor_copy(ones_col[:], ones_col_f[:])
            nc.vector.tensor_copy(ones_row[:], ones_row_f[:])

            # ---- M^T = Wk^T Wq ----
            mt_ps = stp.tile([CIN, GRP * QT], F32, tag="ps", name="mt_ps")
            nc.tensor.matmul(mt_ps[:, :CIN], wk_r[:], wq_r[:], start=True, stop=True)
            mt_r = big.tile([CIN, CIN], F32R)
            nc.vector.tensor_copy(mt_r[:], mt_ps[:, :CIN])

            # ---- Wv^T via identity matmul; stored bf16 for the AV path ----
            wvt_ps = stp.tile([CIN, GRP * QT], F32, tag="ps", name="wvt_ps")
            nc.tensor.matmul(wvt_ps[:, :CIN], wv_r[:], ident_r[:], start=True, stop=True)
            wvt_bf = big.tile([CIN, COUT], BF16)
            nc.vector.tensor_copy(wvt_bf[:], wvt_ps[:, :CIN])

            # ---- Z = M @ x = (M^T)^T x  (f32r) ----
            z_r = big.tile([CIN, N], F32R)
            for j in range(4):
                zp = stp.tile([CIN, GRP * QT], F32, tag="ps", name="zp")
                lo = j * 2 * QT
                nc.tensor.matmul(zp[:, :QT], mt_r[:], xk_r[:, lo: lo + QT],
                                 start=True, stop=True)
                nc.tensor.matmul(zp[:, QT: 2 * QT], mt_r[:],
                                 xk_r[:, lo + QT: lo + 2 * QT],
                                 start=True, stop=True)
                nc.vector.tensor_copy(z_r[:, lo: lo + 2 * QT], zp[:, : 2 * QT])

            # ---- V^T chunks (bf16 matmuls; FWL-fast weight loads) ----
            vt_bf = big.tile([CIN, NMC, MC], BF16)
            for grp in range(NMC // 8):
                vp = stp.tile([CIN, GRP * QT], F32, tag="ps", name="vp")
                for k in range(8):
                    c = grp * 8 + k
                    nc.tensor.matmul(
                        vp[:, k * MC: (k + 1) * MC],
                        xk_bf[:, c * MC: (c + 1) * MC],
                        wvt_bf[:],
                        start=True, stop=True,
                    )
                nc.vector.tensor_copy(vt_bf[:, grp * 8: (grp + 1) * 8, :],
                                      vp[:, : 8 * MC])

            # ---- main loop over query supertiles ----
            groups = []
            c = 0
            while c < NMC:
                cnt = min(GRP, NMC - c)
                groups.append((c, cnt))
                c += cnt

            # den routing: chunks 0-29 spread over 4 accumulators (acc0-2
            # on DVE, acc3 on GpSimd, interleaved so no chain piles up);
            # chunks 30-31 go through PE ones-matmuls at the very end so
            # the end-of-supertile critical path stays short
            GPS_HI = 30

            for st in range(NST):
                q0 = st * QT
                xq_st = xq_r[:, q0: q0 + QT]
                # av[:, :QT] accumulates num[o,q]; av[:, QT:] later holds
                # den broadcast (bank B, written by den-mm then rb-mm)
                av = avp.tile([COUT, 2 * QT], F32, tag="av", name="av")
                accs = [accp.tile([MC, QT], BF16, name=f"acc{a}", tag=f"acc{a}")
                        for a in range(NACC)]
                seen = [0] * NACC
                tail_srcs = []
                for c0, cnt in groups:
                    ps = stp.tile([MC, GRP * QT], F32, tag="ps", name="ps")
                    for k in range(cnt):
                        nc.tensor.matmul(
                            ps[:, k * QT: (k + 1) * QT],
                            z_r[:, (c0 + k) * MC: (c0 + k + 1) * MC],
                            xq_st, start=True, stop=True)
                    pt = ptp.tile([MC, GRP * QT], BF16, tag="pt", name="pt")
                    nc.scalar.activation(pt[:, : cnt * QT], ps[:, : cnt * QT], EXP)
                    for k in range(cnt):
                        cc = c0 + k
                        nc.tensor.matmul(av[:, :QT], vt_bf[:, cc, :],
                                         pt[:, k * QT: (k + 1) * QT],
                                         start=(cc == 0), stop=(cc == NMC - 1))
                    for k in range(cnt):
                        cc = c0 + k
                        src = pt[:, k * QT: (k + 1) * QT]
                        if cc >= GPS_HI:
                            tail_srcs.append(src)
                            continue
                        a = cc % NACC
                        eng = nc.gpsimd if a == NACC - 1 else nc.vector
                        if seen[a] == 0:
                            eng.tensor_copy(accs[a][:], src)
                        else:
                            eng.tensor_tensor(accs[a][:], accs[a][:], src, ADD)
                        seen[a] += 1
                    if c0 + cnt == GPS_HI:
                        # all accumulator chunks are in: combine (acc23 early
                        # on GpSimd, the rest on DVE as chunks land)
                        acc23 = accp.tile([MC, QT], BF16, name="acc23", tag="acc23")
                        nc.gpsimd.tensor_tensor(acc23[:], accs[2][:], accs[3][:], ADD)
                        acc01 = accp.tile([MC, QT], BF16, name="acc01", tag="acc01")
                        nc.vector.tensor_tensor(acc01[:], accs[0][:], accs[1][:], ADD)
                        accf = accp.tile([MC, QT], BF16, name="accf", tag="accf")
                        nc.vector.tensor_tensor(accf[:], acc01[:], acc23[:], ADD)

                # partition-reduce into av bank B: accumulated ones-matmuls
                # (emitted after all S^T/AV work so they don't block it)
                nc.tensor.matmul(av[:1, QT:], ones_col[:], accf[:],
                                 start=True, stop=False)
                for t, src in enumerate(tail_srcs):
                    nc.tensor.matmul(av[:1, QT:], ones_col[:], src,
                                     start=False, stop=(t == len(tail_srcs) - 1))

                # normalize: out = num / den
                den_r = outp.tile([1, QT], F32R, name="den_r")
                nc.vector.tensor_copy(den_r[:], av[:1, QT:])
                # rank-1 matmul broadcasts den over all 128 partitions
                nc.tensor.matmul(av[:, QT:], ones_row[:], den_r[:],
                                 start=True, stop=True)
                rb_sb = outp.tile([COUT, QT], F32, name="rb_sb")
                nc.vector.reciprocal_approx_fast(rb_sb[:], av[:, QT:])
                o_sb = outp.tile([COUT, QT], F32, name="o_sb")
                nc.vector.tensor_tensor(o_sb[:], av[:, :QT], rb_sb[:],
                                        mybir.AluOpType.mult)
                nc.sync.dma_start(out[:, q0: q0 + QT], o_sb[:])

    nc.finalize()
    return nc


_NC_CACHE: list = []
LAST_RESULTS = None


def _get_nc() -> bacc.Bacc:
    if not _NC_CACHE:
        _NC_CACHE.append(_build())
    return _NC_CACHE[0]


def kernel(x, Wq, Wk, Wv, _trace=False):
    global LAST_RESULTS
    x = np.asarray(x, dtype=np.float32)
    wq = np.ascontiguousarray(np.asarray(Wq, dtype=np.float32))
    wk = np.ascontiguousarray(np.asarray(Wk, dtype=np.float32))
    wv = np.ascontiguousarray(np.asarray(Wv, dtype=np.float32))

    nc = _get_nc()
    in_maps = []
    for i in range(NCORES):
        b, h = divmod(i, 2)
        in_maps.append({
            "xq": np.ascontiguousarray(x[b][:, h * NQ: (h + 1) * NQ]),
            "xk": np.ascontiguousarray(x[b]),
            "wq": wq,
            "wk": wk,
            "wv": wv,
        })
    res = run_bass_kernel_spmd(nc, in_maps, core_ids=list(range(NCORES)),
                               trace=_trace)
    LAST_RESULTS = res
    out = np.empty((B, COUT, N), dtype=np.float32)
    for i in range(NCORES):
        b, h = divmod(i, 2)
        out[b][:, h * NQ: (h + 1) * NQ] = res.results[i]["out"]
    return out
